# revision 1
# baseline (speedup 1.0000x reference)
"""Trainium2 Bass kernel for a DiT-style transformer block (AdaLN + attention + SwiGLU MLP).

Sharding: sequence-parallel over 8 cores. Core c owns batch b=c//4, tokens
[512*(c%4), 512*(c%4)+512). K/V are computed in fp8 (k pre-normalized by its
per-token RMS reciprocal) and AllGather'd within each 4-core batch group.
All large GEMMs run in fp8e4 with DoubleRow perf mode (2 contraction planes
per instruction); weights are pre-scaled by powers of two on the host and the
inverse scales are folded into downstream activation scales / gates. PSUM
accumulation stays fp32; the residual stream stays fp32.
"""
import sys
sys.path.insert(0, '/opt/trn_rl_repo')

import numpy as np
import concourse.bass as bass
import concourse.tile as tile
from concourse import bacc, mybir

FP32 = mybir.dt.float32
FP32R = mybir.dt.float32r
FP8 = mybir.dt.float8e4
AF = mybir.ActivationFunctionType
DR = mybir.MatmulPerfMode.DoubleRow

N_CORES = 8
B, T, D, H, DH = 2, 2048, 1024, 16, 64
HM = 2816
TOK = 512            # tokens per core
KT = D // 128        # 8 contraction tiles of 128
KT2 = KT // 2        # 4 DoubleRow tiles of 256
HMT = HM // 128      # 22
HMT2 = HMT // 2      # 11
EPS = 1e-6
EXP_BIAS = -3.0      # exp(s + EXP_BIAS) keeps fp8 outputs in range; cancels in softmax
# host-side power-of-2 weight scales (fp8 dynamic-range placement)
S_QKV = 32.0
S_PROJ = 32.0
S_W1 = 32.0
S_W3 = 4.0
S_W2 = 32.0
S_AW = 256.0
# fp8 kv payload layout (bytes == fp8 elements)
KV_KN = 1024 * TOK              # rope'd, rms-normalized k: [1024 feat, 512 tok]
KV_V = 1040 * TOK               # v (+ones col): [512 tok, 16 heads * 65]
KV_ROW = KV_KN + KV_V           # 1056768 per core
KV_GROUPS = [[0, 1, 2, 3], [4, 5, 6, 7]]


def _ap(t, offset, dims):
    return bass.AP(tensor=t, offset=offset, ap=[list(d) for d in dims])


def build_program(reps=1, nocoll_tail=False):
    nc = bacc.Bacc("TRN2", target_bir_lowering=False, debug=False,
                   num_devices=N_CORES)

    xT = nc.declare_dram_parameter("xT", [D, TOK], FP32, isOutput=False)
    csh = nc.declare_dram_parameter("csh", [D, 1], FP32, isOutput=False)
    # per-core quarter of adaln_w (columns [1536*(core%4), +1536))
    aw = nc.declare_dram_parameter("aw", [D, 6 * D // 4], FP8, isOutput=False)
    ab = nc.declare_dram_parameter("ab", [6 * D], FP32, isOutput=False)
    qkvw = nc.declare_dram_parameter("qkvw", [D, 3 * D], FP8, isOutput=False)
    projw = nc.declare_dram_parameter("projw", [D, D], FP8, isOutput=False)
    projb = nc.declare_dram_parameter("projb", [D], FP32, isOutput=False)
    w1 = nc.declare_dram_parameter("w1", [D, HM], FP8, isOutput=False)
    w3 = nc.declare_dram_parameter("w3", [D, HM], FP8, isOutput=False)
    w2 = nc.declare_dram_parameter("w2", [HM, D], FP8, isOutput=False)
    BF16 = mybir.dt.bfloat16
    cosq = nc.declare_dram_parameter("cosq", [128, TOK], BF16, isOutput=False)
    sinq = nc.declare_dram_parameter("sinq", [128, TOK], BF16, isOutput=False)
    cosk = nc.declare_dram_parameter("cosk", [128, TOK], BF16, isOutput=False)
    sink = nc.declare_dram_parameter("sink", [128, TOK], BF16, isOutput=False)
    perm = nc.declare_dram_parameter("perm", [128, 128], FP32, isOutput=False)
    consts = nc.declare_dram_parameter("consts", [128, 4], FP32, isOutput=False)
    outT = nc.declare_dram_parameter("outT", [D, TOK], FP32, isOutput=True)

    io = locals()
    with tile.TileContext(nc) as tc:
        for _rep in range(reps):
            _body(nc, tc, io, skip_collectives=(nocoll_tail and _rep > 0))
    nc.compile()
    return nc


def _body(nc, tc, io, skip_collectives=False):
    xT, csh, aw, ab = io["xT"], io["csh"], io["aw"], io["ab"]
    qkvw, projw, projb = io["qkvw"], io["projw"], io["projb"]
    w1, w3, w2 = io["w1"], io["w3"], io["w2"]
    cosq, sinq, cosk, sink = io["cosq"], io["sinq"], io["cosk"], io["sink"]
    perm, consts, outT = io["perm"], io["consts"], io["outT"]

    from contextlib import ExitStack
    ctx = ExitStack()
    # pools alive for the whole kernel
    pp = ctx.enter_context(tc.tile_pool(name="persist", bufs=1))
    dr = ctx.enter_context(tc.tile_pool(name="dram", bufs=1, space="DRAM"))

    # ---------- global constants ----------
    consts_sb = pp.tile([128, 4], FP32R, name="consts_sb")
    nc.sync.dma_start(out=consts_sb, in_=consts[:, :].bitcast(FP32R))
    xT_sb = pp.tile([128, KT, TOK], FP32, name="xT_sb")
    nc.sync.dma_start(out=xT_sb, in_=xT.rearrange("(kt p) t -> p kt t", p=128))
    projb_sb = pp.tile([128, 8], FP32, name="projb_sb")
    nc.sync.dma_start(out=projb_sb, in_=projb.rearrange("(oc p) -> p oc", p=128))
    eps1 = pp.tile([128, 1], FP32, name="eps1")
    nc.vector.memset(eps1, EPS)
    epsd = pp.tile([128, 1], FP32, name="epsd")
    nc.vector.memset(epsd, DH * EPS)
    ebias = pp.tile([128, 1], FP32, name="ebias")
    nc.vector.memset(ebias, EXP_BIAS)
    mods_sb = pp.tile([128, 6, 8], FP32, name="mods_sb")
    s1p_msa = pp.tile([128, 8], FP32, name="s1p_msa")
    s1p_mlp = pp.tile([128, 8], FP32, name="s1p_mlp")
    bgp = pp.tile([128, 8], FP32, name="bgp")
    gproj = pp.tile([128, 8], FP32, name="gproj")   # gate_msa / (S_QKV*S_PROJ/32)
    gmlp = pp.tile([128, 8], FP32, name="gmlp")     # gate_mlp / (S_W3*S_W2)
    x1T = pp.tile([128, KT, TOK], FP32, name="x1T")
    perm_sb = pp.tile([128, 128], FP32R, name="perm_sb")
    nc.sync.dma_start(out=perm_sb, in_=perm[:, :].bitcast(FP32R))

    # ---------- DRAM scratch ----------
    mods_q = dr.tile([6 * D // 4], FP32, name="mods_q")
    mods_in = dr.tile([6 * D], FP32, name="mods_in")
    kv_in = dr.tile([KV_ROW], FP8, name="kv_in")
    kv_out = dr.tile([4, KV_ROW], FP8, name="kv_out")

    def rms_rb(pool_ps, pool_t, pool_s, src_tile3, scale, bias, tag):
        """token-wise 1/sqrt(mean(sq)+eps) over the partition (feature) dim,
        broadcast to all 128 partitions (via DRAM round-trip; keeps the Pool
        engine on the `standard` library -> no ucode reloads)"""
        ps_ssq = pool_ps.tile([1, 512], FP32, tag=f"ps_ssq_{tag}", bufs=1,
                              name=f"ps_ssq_{tag}")
        for kt in range(KT):
            xsq = pool_t.tile([128, TOK], FP32R, tag=f"sq_{tag}", bufs=2,
                              name=f"sq_{tag}")
            nc.gpsimd.tensor_mul(xsq, src_tile3[:, kt, :], src_tile3[:, kt, :])
            nc.tensor.matmul(ps_ssq, consts_sb[:, 0:1], xsq,
                             start=(kt == 0), stop=(kt == KT - 1))
        rt = pool_s.tile([1, 512], FP32, tag=f"rt_{tag}", name=f"rt_{tag}")
        nc.scalar.activation(out=rt, in_=ps_ssq, func=AF.Sqrt,
                             scale=scale, bias=bias)
        rv = pool_s.tile([1, 512], FP32, tag=f"rv_{tag}", name=f"rv_{tag}")
        nc.vector.reciprocal(rv, rt)
        rvd = dr.tile([1, 512], FP32, tag=f"rvd_{tag}", bufs=2,
                      name=f"rvd_{tag}")
        nc.sync.dma_start(out=rvd, in_=rv)
        rb = pool_t.tile([128, TOK], FP32, tag=f"rb_{tag}", bufs=1,
                         name=f"rb_{tag}")
        nc.sync.dma_start(
            out=rb, in_=_ap(rvd.tensor, rvd.offset, [[0, 128], [1, 512]]))
        return rb

    # =========================================================
    # Scope AB: qn lives from P3 into attention
    # =========================================================
    ab_pool = ctx.enter_context(tc.tile_pool(name="scope_ab", bufs=1))
    qn = ab_pool.tile([128, 8, TOK], FP8, name="qn")

    with tc.tile_pool(name="sA", bufs=1) as pa, \
         tc.tile_pool(name="wA", bufs=2) as wp, \
         tc.tile_pool(name="tA", bufs=2) as tp, \
         tc.tile_pool(name="smA", bufs=1) as sp:

        # ---------- P0: AdaLN mods ----------
        with tc.tile_pool(name="psA1", bufs=1, space="PSUM") as ps1:
            csh_sb = pa.tile([128, 8, 1], FP32, name="csh_sb")
            nc.sync.dma_start(out=csh_sb,
                              in_=csh.rearrange("(kt p) o -> p kt o", p=128))
            # fp8 silu(c), padded so the DoubleRow plane stride is 16B
            silu_c = pa.tile([128, 8, 16], FP8, name="silu_c")
            nc.scalar.activation(out=silu_c[:, :, 0:1], in_=csh_sb,
                                 func=AF.Silu)
            for ncn in range(3):
                aw_t = wp.tile([128, KT2, 2, 512], FP8, tag="aw_t", name="aw_t")
                nc.sync.dma_start(
                    out=aw_t,
                    in_=aw[:, 512 * ncn:512 * (ncn + 1)]
                    .rearrange("(kt2 two p) n -> p kt2 two n", p=128, two=2))
                ps_m = ps1.tile([1, 512], FP32, tag="ps_mods", bufs=2,
                                name="ps_m")
                for kt2 in range(KT2):
                    nc.tensor.matmul(ps_m, silu_c[:, 2 * kt2:2 * kt2 + 2, 0:1],
                                     aw_t[:, kt2, :, :],
                                     start=(kt2 == 0), stop=(kt2 == KT2 - 1),
                                     perf_mode=DR)
                stg = sp.tile([1, 512], FP32, tag="mods_stg", name="stg")
                nc.vector.tensor_scalar_mul(stg, ps_m, 1.0 / S_AW)
                nc.sync.dma_start(
                    out=_ap(mods_q.tensor, mods_q.offset + 512 * ncn,
                            [[512, 1], [1, 512]]),
                    in_=stg)
            if not skip_collectives:
                nc.gpsimd.collective_compute(
                    "AllGather", mybir.AluOpType.bypass,
                    replica_groups=KV_GROUPS,
                    ins=[mods_q[:]],
                    outs=[mods_in[:]])
            nc.sync.dma_start(
                out=mods_sb,
                in_=mods_in.rearrange("(v kt p) -> p v kt", p=128, kt=8))
            ab_sb = pa.tile([128, 6, 8], FP32, name="ab_sb")
            nc.sync.dma_start(
                out=ab_sb, in_=ab.rearrange("(v kt p) -> p v kt", p=128, kt=8))
            nc.vector.tensor_add(mods_sb, mods_sb, ab_sb)
            nc.scalar.add(s1p_msa, mods_sb[:, 1, :], 1.0)
            nc.scalar.add(s1p_mlp, mods_sb[:, 4, :], 1.0)
            nc.vector.tensor_mul(bgp, projb_sb, mods_sb[:, 2, :])
            nc.vector.tensor_scalar_mul(gproj, mods_sb[:, 2, :],
                                        1.0 / (S_QKV * S_PROJ / S_QKV))
            nc.vector.tensor_scalar_mul(gmlp, mods_sb[:, 5, :],
                                        1.0 / (S_W3 * S_W2))

            # ---------- P1: x_modT (fp8) ----------
            rb1 = rms_rb(ps1, tp, sp, xT_sb, 1.0 / D, eps1[0:1, :], "n1")
            x_modT = pa.tile([128, KT, TOK], FP8, name="x_modT")
            for kt in range(KT):
                xr = tp.tile([128, TOK], FP32, tag="xr1", name="xr")
                nc.gpsimd.tensor_mul(xr, xT_sb[:, kt, :], rb1)
                nc.vector.tensor_scalar(
                    out=x_modT[:, kt, :], in0=xr,
                    scalar1=s1p_msa[:, kt:kt + 1],
                    scalar2=mods_sb[:, 0, kt:kt + 1],
                    op0=mybir.AluOpType.mult, op1=mybir.AluOpType.add)

        # ---------- P2/P3: q/k/v projections, rope, kv allgather ----------
        BF16 = mybir.dt.bfloat16
        cosq_sb = pa.tile([128, TOK], BF16, name="cosq_sb")
        sinq_sb = pa.tile([128, TOK], BF16, name="sinq_sb")
        cosk_sb = pa.tile([128, TOK], BF16, name="cosk_sb")
        sink_sb = pa.tile([128, TOK], BF16, name="sink_sb")
        nc.sync.dma_start(out=cosq_sb, in_=cosq[:, :])
        nc.sync.dma_start(out=sinq_sb, in_=sinq[:, :])
        nc.sync.dma_start(out=cosk_sb, in_=cosk[:, :])
        nc.sync.dma_start(out=sink_sb, in_=sink[:, :])

        with tc.tile_pool(name="psA2", bufs=1, space="PSUM") as ps2:

            def proj_T(col0, oc):
                w_t = wp.tile([128, KT2, 2, 128], FP8, tag="w_pT", name="w_t")
                nc.sync.dma_start(
                    out=w_t,
                    in_=qkvw[:, col0 + 128 * oc: col0 + 128 * (oc + 1)]
                    .rearrange("(kt2 two p) n -> p kt2 two n", p=128, two=2))
                ps_p = ps2.tile([128, 512], FP32, tag="ps_pT", bufs=2,
                                name="ps_p")
                for kt2 in range(KT2):
                    nc.tensor.matmul(ps_p, w_t[:, kt2, :, :],
                                     x_modT[:, 2 * kt2:2 * kt2 + 2, :],
                                     start=(kt2 == 0), stop=(kt2 == KT2 - 1),
                                     perf_mode=DR)
                return ps_p

            def rope(ps_raw, cos_sb, sin_sb, tag):
                raw = tp.tile([128, TOK], FP32R, tag=f"raw_{tag}", bufs=2,
                              name="raw")
                nc.vector.tensor_copy(out=raw, in_=ps_raw)
                ps_sh = ps2.tile([128, 512], FP32, tag="ps_sh", bufs=2,
                                 name="ps_sh")
                nc.tensor.matmul(ps_sh, perm_sb, raw, start=True, stop=True)
                t1 = tp.tile([128, TOK], FP32, tag=f"t1_{tag}", bufs=2,
                             name="t1")
                nc.gpsimd.tensor_mul(t1, raw, cos_sb)
                t2 = tp.tile([128, TOK], FP32, tag=f"t2_{tag}", bufs=2,
                             name="t2")
                nc.vector.tensor_mul(t2, ps_sh, sin_sb)
                return raw, t1, t2

            def head_rms(raw, scale, bias, tag):
                sq = tp.tile([128, TOK], FP32R, tag=f"hsq_{tag}", bufs=2,
                             name="sq")
                nc.gpsimd.tensor_mul(sq, raw, raw)
                ps_h = ps2.tile([2, 512], FP32, tag="ps_h", bufs=2,
                                name="ps_h")
                nc.tensor.matmul(ps_h, consts_sb[:, 1:3], sq,
                                 start=True, stop=True)
                rs = sp.tile([2, 512], FP32, tag=f"rs_{tag}", name="rs")
                nc.scalar.activation(out=rs, in_=ps_h, func=AF.Sqrt,
                                     scale=scale, bias=bias)
                rvv = sp.tile([2, 512], FP32, tag=f"rvv_{tag}", name="rvv")
                nc.vector.reciprocal(rvv, rs)
                return rvv

            def bc_pair(rvv, tag):
                # [2,512] per-head scalars -> [128,512] via DRAM round-trip
                rd2 = dr.tile([2, 512], FP32, tag=f"rd2_{tag}", bufs=2,
                              name=f"rd2_{tag}")
                nc.sync.dma_start(out=rd2, in_=rvv)
                rb2 = tp.tile([128, TOK], FP32, tag=f"rb2_{tag}", bufs=2,
                              name=f"rb2_{tag}")
                nc.sync.dma_start(
                    out=rb2[0:64, :],
                    in_=_ap(rd2.tensor, rd2.offset, [[0, 64], [1, 512]]))
                nc.sync.dma_start(
                    out=rb2[64:128, :],
                    in_=_ap(rd2.tensor, rd2.offset + 512, [[0, 64], [1, 512]]))
                return rb2

            for oc in range(8):
                ps_k = proj_T(D, oc)
                raw, t1, t2 = rope(ps_k, cosk_sb, sink_sb, "k")
                rkv = head_rms(raw, 1.0 / DH, eps1[0:2, :], "k")
                rkb = bc_pair(rkv, "k")
                knt = tp.tile([128, TOK], FP32, tag="knt", bufs=2, name="knt")
                nc.gpsimd.tensor_add(knt, t1, t2)
                kn8 = tp.tile([128, TOK], FP8, tag="kn8", bufs=2, name="kn8")
                nc.vector.tensor_mul(kn8, knt, rkb)
                nc.sync.dma_start(
                    out=_ap(kv_in.tensor, kv_in.offset + 128 * oc * 512,
                            [[512, 128], [1, 512]]),
                    in_=kn8)

            for ncn in range(2):
                wv_t = wp.tile([128, KT2, 2, 512], FP8, tag="wv_t", bufs=1,
                               name="wv_t")
                nc.sync.dma_start(
                    out=wv_t,
                    in_=qkvw[:, 2 * D + 512 * ncn: 2 * D + 512 * (ncn + 1)]
                    .rearrange("(kt2 two p) n -> p kt2 two n", p=128, two=2))
                for mt in range(4):
                    vaug = tp.tile([128, 8, 65], FP8, tag="vaug", bufs=2,
                                   name="vaug")
                    nc.vector.memset(vaug[:, :, 64:65], 1.0)
                    ps_v = ps2.tile([128, 512], FP32, tag="ps_pT", bufs=2,
                                    name="ps_v")
                    for kt2 in range(KT2):
                        nc.tensor.matmul(
                            ps_v,
                            x_modT[:, 2 * kt2:2 * kt2 + 2,
                                   128 * mt:128 * (mt + 1)],
                            wv_t[:, kt2, :, :],
                            start=(kt2 == 0), stop=(kt2 == KT2 - 1),
                            perf_mode=DR)
                    nc.vector.tensor_scalar_mul(
                        vaug[:, :, 0:64],
                        ps_v.rearrange("p (h d) -> p h d", d=64),
                        1.0 / S_QKV)
                    nc.sync.dma_start(
                        out=_ap(kv_in.tensor,
                                kv_in.offset + KV_KN + 128 * mt * 1040
                                + 65 * 8 * ncn,
                                [[1040, 128], [1, 520]]),
                        in_=vaug.rearrange("p h d -> p (h d)"))

            if not skip_collectives:
                nc.gpsimd.collective_compute(
                    "AllGather", mybir.AluOpType.bypass,
                    replica_groups=KV_GROUPS,
                    ins=[kv_in[:]],
                    outs=[kv_out.rearrange("a b -> (a b)")])

            # ---------- P3: qT ----------
            for oc in range(8):
                ps_q = proj_T(0, oc)
                raw, t1, t2 = rope(ps_q, cosq_sb, sinq_sb, "q")
                rqv = head_rms(raw, 1.0, epsd[0:2, :], "q")
                rqb = bc_pair(rqv, "q")
                t3 = tp.tile([128, TOK], FP32, tag="t3_q", bufs=2, name="t3")
                nc.gpsimd.tensor_add(t3, t1, t2)
                nc.vector.tensor_mul(qn[:, oc, :], t3, rqb)

    # =========================================================
    # Scope B: attention + proj
    # =========================================================
    with tc.tile_pool(name="sB", bufs=1) as pb, \
         tc.tile_pool(name="wB", bufs=2) as wpb, \
         tc.tile_pool(name="tB", bufs=2) as tpb, \
         tc.tile_pool(name="smB", bufs=1) as spb, \
         tc.tile_pool(name="psB", bufs=1, space="PSUM") as psb:

        attn_all = pb.tile([64, H, TOK], FP8, name="attn_all")

        kv_t = kv_out.tensor
        kv_off = kv_out.offset
        for hp in range(8):
            kn_pair = wpb.tile([128, 4, 512], FP8, tag="kn_pair", bufs=3,
                               name="kn_pair")
            nc.sync.dma_start(
                out=kn_pair,
                in_=_ap(kv_t, kv_off + 128 * hp * 512,
                        [[512, 128], [KV_ROW, 4], [1, 512]]))
            vaug_h = []
            for hh in range(2):
                h = 2 * hp + hh
                # padded to 80 cols so the DoubleRow plane stride is 16B-aligned
                vh = wpb.tile([128, 4, 4, 80], FP8, tag=f"vaug_h{hh}",
                              bufs=3, name=f"vaug_h{hh}")
                for s2 in range(4):
                    nc.sync.dma_start(
                        out=vh[:, s2, :, 0:65],
                        in_=_ap(kv_t, kv_off + s2 * KV_ROW + KV_KN + 65 * h,
                                [[1040, 128], [128 * 1040, 4], [1, 65]]))
                vaug_h.append(vh)
            ps_o = []
            for hh in range(2):
                pso = psb.tile([65, 512], FP32, tag=f"ps_o{hh}", bufs=1,
                               name=f"ps_o{hh}")
                ps_o.append(pso)
            for sp_i in range(8):
                s, i2 = sp_i // 2, sp_i % 2
                exp8 = []
                for hh in range(2):
                    e8 = tpb.tile([128, 2, 512], FP8, tag=f"exp{hh}", bufs=2,
                                  name=f"exp{hh}")
                    exp8.append(e8)
                for j in range(2):
                    u = 2 * i2 + j
                    for hh in range(2):
                        ps_s = psb.tile([128, 512], FP32, tag=f"ps_s{hh}",
                                        bufs=2, name=f"ps_s{hh}")
                        nc.tensor.matmul(
                            ps_s,
                            kn_pair[64 * hh:64 * (hh + 1), s,
                                    128 * u:128 * (u + 1)],
                            qn[64 * hh:64 * (hh + 1), hp, :],
                            start=True, stop=True, tile_position=(64 * hh, 0))
                        nc.scalar.activation(out=exp8[hh][:, j, :], in_=ps_s,
                                             func=AF.Exp, bias=ebias[:, :])
                for hh in range(2):
                    nc.tensor.matmul(ps_o[hh],
                                     vaug_h[hh][:, s, 2 * i2:2 * i2 + 2, 0:65],
                                     exp8[hh],
                                     start=(sp_i == 0), stop=(sp_i == 7),
                                     perf_mode=DR)
            for hh in range(2):
                h = 2 * hp + hh
                rd = spb.tile([1, 512], FP32, tag=f"rd{hh}", name=f"rd{hh}")
                nc.vector.reciprocal(rd, ps_o[hh][64:65, :])
                rdd = dr.tile([1, 512], FP32, tag=f"rdd{hh}", bufs=2,
                              name=f"rdd{hh}")
                nc.sync.dma_start(out=rdd, in_=rd)
                rdb = tpb.tile([64, 512], FP32, tag=f"rdb{hh}", bufs=2,
                               name=f"rdb{hh}")
                nc.sync.dma_start(
                    out=rdb,
                    in_=_ap(rdd.tensor, rdd.offset, [[0, 64], [1, 512]]))
                nc.vector.tensor_mul(attn_all[:, h, :], ps_o[hh][0:64, :], rdb)

        # ---------- P5: proj + gated residual -> x1T ----------
        for oc in range(8):
            wproj_t = wpb.tile([64, 8, 2, 128], FP8, tag="wproj_t", bufs=3,
                               name="wproj_t")
            nc.sync.dma_start(
                out=wproj_t,
                in_=projw[:, 128 * oc:128 * (oc + 1)]
                .rearrange("(hp two p) m -> p hp two m", p=64, two=2))
            ps_p = psb.tile([128, 512], FP32, tag="ps_proj", bufs=2,
                            name="ps_p")
            for hp in range(8):
                nc.tensor.matmul(ps_p, wproj_t[:, hp, :, :],
                                 attn_all[:, 2 * hp:2 * hp + 2, :],
                                 start=(hp == 0), stop=(hp == 7),
                                 perf_mode=DR)
            t1 = tpb.tile([128, TOK], FP32, tag="t1_proj", bufs=2, name="t1")
            nc.vector.tensor_scalar(
                out=t1, in0=ps_p,
                scalar1=gproj[:, oc:oc + 1], scalar2=bgp[:, oc:oc + 1],
                op0=mybir.AluOpType.mult, op1=mybir.AluOpType.add)
            nc.gpsimd.tensor_add(x1T[:, oc, :], t1, xT_sb[:, oc, :])

    # =========================================================
    # Scope C: norm2 + MLP
    # =========================================================
    with tc.tile_pool(name="sC", bufs=1) as pc, \
         tc.tile_pool(name="wC", bufs=2) as wpc, \
         tc.tile_pool(name="tC", bufs=2) as tpc, \
         tc.tile_pool(name="smC", bufs=1) as spc, \
         tc.tile_pool(name="psC", bufs=1, space="PSUM") as psc:

        rb2 = rms_rb(psc, tpc, spc, x1T, 1.0 / D, eps1[0:1, :], "n2")
        x1_modT = pc.tile([128, KT, TOK], FP8, name="x1_modT")
        for kt in range(KT):
            xr2 = tpc.tile([128, TOK], FP32, tag="xr2", name="xr2")
            nc.gpsimd.tensor_mul(xr2, x1T[:, kt, :], rb2)
            nc.vector.tensor_scalar(
                out=x1_modT[:, kt, :], in0=xr2,
                scalar1=s1p_mlp[:, kt:kt + 1],
                scalar2=mods_sb[:, 3, kt:kt + 1],
                op0=mybir.AluOpType.mult, op1=mybir.AluOpType.add)

        mT = pc.tile([128, HMT, TOK], FP8, name="mT")
        for hm in range(HMT):
            w1_t = wpc.tile([128, KT2, 2, 128], FP8, tag="w1_t", bufs=3,
                            name="w1_t")
            nc.sync.dma_start(
                out=w1_t,
                in_=w1[:, 128 * hm:128 * (hm + 1)]
                .rearrange("(kt2 two p) n -> p kt2 two n", p=128, two=2))
            w3_t = wpc.tile([128, KT2, 2, 128], FP8, tag="w3_t", bufs=3,
                            name="w3_t")
            nc.sync.dma_start(
                out=w3_t,
                in_=w3[:, 128 * hm:128 * (hm + 1)]
                .rearrange("(kt2 two p) n -> p kt2 two n", p=128, two=2))
            ps_u = psc.tile([128, 512], FP32, tag="ps_u", bufs=2, name="ps_u")
            ps_g = psc.tile([128, 512], FP32, tag="ps_g", bufs=2, name="ps_g")
            for kt2 in range(KT2):
                nc.tensor.matmul(ps_u, w1_t[:, kt2, :, :],
                                 x1_modT[:, 2 * kt2:2 * kt2 + 2, :],
                                 start=(kt2 == 0), stop=(kt2 == KT2 - 1),
                                 perf_mode=DR)
            for kt2 in range(KT2):
                nc.tensor.matmul(ps_g, w3_t[:, kt2, :, :],
                                 x1_modT[:, 2 * kt2:2 * kt2 + 2, :],
                                 start=(kt2 == 0), stop=(kt2 == KT2 - 1),
                                 perf_mode=DR)
            tsil = tpc.tile([128, TOK], FP32, tag="tsil", name="tsil")
            nc.scalar.activation(out=tsil, in_=ps_u, func=AF.Silu,
                                 scale=1.0 / S_W1)
            nc.vector.tensor_mul(mT[:, hm, :], tsil, ps_g)
            # (kept on DVE: ps_g is PSUM; Pool PSUM access is unproven)

        for oc in range(8):
            w2_t = wpc.tile([128, HMT2, 2, 128], FP8, tag="w2_t", bufs=3,
                            name="w2_t")
            nc.sync.dma_start(
                out=w2_t,
                in_=w2[:, 128 * oc:128 * (oc + 1)]
                .rearrange("(hm2 two p) n -> p hm2 two n", p=128, two=2))
            ps_w2 = psc.tile([128, 512], FP32, tag="ps_w2", bufs=2,
                             name="ps_w2")
            for hm2 in range(HMT2):
                nc.tensor.matmul(ps_w2, w2_t[:, hm2, :, :],
                                 mT[:, 2 * hm2:2 * hm2 + 2, :],
                                 start=(hm2 == 0), stop=(hm2 == HMT2 - 1),
                                 perf_mode=DR)
            t3 = tpc.tile([128, TOK], FP32, tag="t3_out", bufs=2, name="t3")
            nc.vector.tensor_scalar(
                out=t3, in0=ps_w2,
                scalar1=gmlp[:, oc:oc + 1], scalar2=None,
                op0=mybir.AluOpType.mult)
            outf = tpc.tile([128, TOK], FP32, tag="outf", bufs=2, name="outf")
            nc.gpsimd.tensor_add(outf, t3, x1T[:, oc, :])
            nc.sync.dma_start(out=outT[128 * oc:128 * (oc + 1), :], in_=outf)

    ctx.close()


# ------------------------------------------------------------------
# host side
# ------------------------------------------------------------------

def _to_fp8(a, scale):
    import ml_dtypes
    return np.clip(np.asarray(a, np.float32) * scale, -240.0, 240.0).astype(
        ml_dtypes.float8_e4m3)


def _host_tables(pos, lnq_w, lnk_w):
    half = DH // 2
    freqs = (1.0 / (10000.0 ** (np.arange(half, dtype=np.float32) / half))
             ).astype(np.float32)
    ang = pos.astype(np.float32)[:, None] * freqs[None, :]      # [T, 32]
    cos2 = np.concatenate([np.cos(ang), np.cos(ang)], -1).astype(np.float32)
    sin2 = np.concatenate([np.sin(ang), np.sin(ang)], -1).astype(np.float32)
    shufsrc = np.concatenate([np.arange(32) + 32, np.arange(32)])
    cosF_q = cos2 * lnq_w[None, :]
    sinF_q = sin2 * lnq_w[shufsrc][None, :]
    cosF_k = cos2 * lnk_w[None, :]
    sinF_k = sin2 * lnk_w[shufsrc][None, :]

    P = np.zeros((128, 128), np.float32)
    for blk in (0, 64):
        for m in range(64):
            P[blk + shufsrc[m], blk + m] = -1.0 if m < 32 else 1.0

    consts = np.zeros((128, 4), np.float32)
    consts[:, 0] = 1.0
    consts[0:64, 1] = 1.0
    consts[64:128, 2] = 1.0
    return cosF_q, sinF_q, cosF_k, sinF_k, P, consts


def _prep_in_maps(inputs):
    x = np.asarray(inputs["x"], np.float32)
    c = np.asarray(inputs["c"], np.float32)
    pos = np.asarray(inputs["pos"])
    cosF_q, sinF_q, cosF_k, sinF_k, P, consts = _host_tables(
        pos, np.asarray(inputs["lnq_w"], np.float32),
        np.asarray(inputs["lnk_w"], np.float32))
    shared = {
        "ab": np.ascontiguousarray(inputs["adaln_b"], np.float32),
        "qkvw": _to_fp8(inputs["qkv_w"], S_QKV),
        "projw": _to_fp8(inputs["proj_w"], S_PROJ),
        "projb": np.ascontiguousarray(inputs["proj_b"], np.float32),
        "w1": _to_fp8(inputs["w1_w"], S_W1),
        "w3": _to_fp8(inputs["w3_w"], S_W3),
        "w2": _to_fp8(inputs["w2_w"], S_W2),
        "perm": P, "consts": consts,
    }
    aw8 = _to_fp8(inputs["adaln_w"], S_AW)
    in_maps = []
    for core in range(N_CORES):
        b, ti = core // 4, core % 4
        q0 = TOK * ti
        import ml_dtypes
        tile2 = lambda a: np.ascontiguousarray(
            np.tile(a[q0:q0 + TOK].T, (2, 1))).astype(
                ml_dtypes.bfloat16)  # [64,512] -> [128,512]
        m = dict(shared)
        m["xT"] = np.ascontiguousarray(x[b, q0:q0 + TOK, :].T)
        m["csh"] = np.ascontiguousarray(c[b]).reshape(D, 1)
        m["aw"] = np.ascontiguousarray(
            aw8[:, 1536 * ti:1536 * (ti + 1)])
        m["cosq"] = tile2(cosF_q)
        m["sinq"] = tile2(sinF_q)
        m["cosk"] = tile2(cosF_k)
        m["sink"] = tile2(sinF_k)
        in_maps.append(m)
    return in_maps


_RUNNER = {}


def _get_runner(reps=1, nocoll_tail=False):
    global _RUNNER
    key = (reps, nocoll_tail)
    if key in _RUNNER:
        return _RUNNER[key]
    import jax
    from jax.sharding import Mesh, PartitionSpec
    from jax.experimental.shard_map import shard_map
    from concourse import bass2jax, mybir as _mybir

    nc = build_program(reps, nocoll_tail)
    bass2jax.install_neuronx_cc_hook()

    partition_name = (nc.partition_id_tensor.name
                      if nc.partition_id_tensor else None)
    in_names, out_names, out_avals, zero_outs = [], [], [], []
    for alloc in nc.m.functions[0].allocations:
        if not isinstance(alloc, _mybir.MemoryLocationSet):
            continue
        name = alloc.memorylocations[0].name
        if alloc.kind == "ExternalInput":
            if name != partition_name:
                in_names.append(name)
        elif alloc.kind == "ExternalOutput":
            shape = tuple(alloc.tensor_shape)
            dtype = _mybir.dt.np(alloc.dtype)
            out_names.append(name)
            out_avals.append(jax.core.ShapedArray(shape, dtype))
            zero_outs.append(np.zeros(shape, dtype))
    n_params = len(in_names)
    n_outs = len(out_avals)
    all_names = in_names + out_names
    if partition_name is not None:
        all_names = all_names + [partition_name]
    donate = tuple(range(n_params, n_params + n_outs))

    def _bd(*args):
        operands = list(args)
        if partition_name is not None:
            operands.append(bass2jax.partition_id_tensor())
        outs = bass2jax._bass_exec_p.bind(
            *operands, out_avals=tuple(out_avals), in_names=tuple(all_names),
            out_names=tuple(out_names), lowering_input_output_aliases=(),
            sim_require_finite=True, sim_require_nnan=True, nc=nc)
        return tuple(outs)

    devices = jax.devices()[:N_CORES]
    mesh = Mesh(np.asarray(devices), ("core",))
    sharded = jax.jit(
        shard_map(_bd, mesh=mesh,
                  in_specs=(PartitionSpec("core"),) * (n_params + n_outs),
                  out_specs=(PartitionSpec("core"),) * n_outs,
                  check_rep=False),
        donate_argnums=donate, keep_unused=True)

    def run(in_maps):
        concat_in = [np.concatenate([np.asarray(m[nm]) for m in in_maps], 0)
                     for nm in in_names]
        concat_zeros = [np.zeros((N_CORES * z.shape[0], *z.shape[1:]), z.dtype)
                        for z in zero_outs]
        out_arrs = sharded(*concat_in, *concat_zeros)
        return [
            {nm: np.asarray(out_arrs[i]).reshape(N_CORES, *out_avals[i].shape)[cc]
             for i, nm in enumerate(out_names)}
            for cc in range(N_CORES)
        ]

    def bench(in_maps, iters_lo=4, iters_hi=24):
        import time as _time
        concat_in = [np.concatenate([np.asarray(m[nm]) for m in in_maps], 0)
                     for nm in in_names]
        dev_in = [jax.device_put(a) for a in concat_in]
        for a in dev_in:
            a.block_until_ready()

        def zero_set():
            zs = [jax.device_put(
                np.zeros((N_CORES * z.shape[0], *z.shape[1:]), z.dtype))
                for z in zero_outs]
            for z in zs:
                z.block_until_ready()
            return zs

        out = sharded(*dev_in, *zero_set())  # warm-up
        for o in out:
            o.block_until_ready()

        results = {}
        for iters in (iters_lo, iters_hi):
            staged = [zero_set() for _ in range(iters)]
            t0 = _time.time()
            out = None
            for i in range(iters):
                out = sharded(*dev_in, *staged[i])
            for o in out:
                o.block_until_ready()
            results[iters] = _time.time() - t0
        per_iter = (results[iters_hi] - results[iters_lo]) / (iters_hi - iters_lo)
        return per_iter, results

    run.bench = bench
    run.sharded = sharded
    run.zero_outs = zero_outs
    run.in_names = in_names
    _RUNNER[key] = run
    return run


def kernel(**inputs) -> np.ndarray:
    run = _get_runner()
    in_maps = _prep_in_maps(inputs)
    results = run(in_maps)
    out = np.empty((B, T, D), np.float32)
    for core in range(N_CORES):
        b, ti = core // 4, core % 4
        out[b, TOK * ti:TOK * (ti + 1), :] = results[core]["outT"].T
    return out



# revision 36
# speedup vs baseline: 1.0185x; 1.0185x over previous
"""Trainium2 Bass kernel for a DiT-style transformer block (AdaLN + attention + SwiGLU MLP).

Sharding: sequence-parallel over 8 cores. Core c owns batch b=c//4, tokens
[512*(c%4), 512*(c%4)+512). K/V are computed in fp8 (k pre-normalized by its
per-token RMS reciprocal) and AllGather'd within each 4-core batch group.
All large GEMMs run in fp8e4 with DoubleRow perf mode (2 contraction planes
per instruction); weights are pre-scaled by powers of two on the host and the
inverse scales are folded into downstream activation scales / gates. PSUM
accumulation stays fp32; the residual stream stays fp32.

Structure (vs the earlier draft): all weights preloaded whole into SBUF at
body start (contiguous-row DMAs spread across the sync/scalar/gpsimd DGE
queues); per-token scalars (1/rms, 1/softmax-denominator) broadcast to all
partitions with tiny fp32r ones-matmuls on the PE instead of DRAM
round-trips; RMS chains fused into single ACT-engine Rsqrt instructions;
v-projection chunks interleaved between k rope/norm chains to keep the PE
warm; attention loads all heads' v once ([tok,src,blk,16*65] tile) and lags
the PV accumulation one step behind the exps so it never parks the in-order
PE queue; rope temporaries in bf16.
"""
import sys
sys.path.insert(0, '/opt/trn_rl_repo')

import numpy as np
import concourse.bass as bass
import concourse.tile as tile
from concourse import bacc, mybir

FP32 = mybir.dt.float32
FP32R = mybir.dt.float32r
FP8 = mybir.dt.float8e4
BF16 = mybir.dt.bfloat16
AF = mybir.ActivationFunctionType
DR = mybir.MatmulPerfMode.DoubleRow

N_CORES = 8
B, T, D, H, DH = 2, 2048, 1024, 16, 64
HM = 2816
TOK = 512            # tokens per core
KT = D // 128        # 8 contraction tiles of 128
KT2 = KT // 2        # 4 DoubleRow tiles of 256
HMT = HM // 128      # 22
HMT2 = HMT // 2      # 11
EPS = 1e-6
EXP_BIAS = -3.0      # exp(s + EXP_BIAS) keeps fp8 outputs in range; cancels in softmax
# host-side power-of-2 weight scales (fp8 dynamic-range placement)
S_QKV = 32.0
S_PROJ = 32.0
S_W1 = 32.0
S_W3 = 4.0
S_W2 = 32.0
S_AW = 256.0
# fp8 kv payload layout (bytes == fp8 elements)
KV_KN = 1024 * TOK              # rope'd, rms-normalized k: [1024 feat, 512 tok]
KV_V = 1040 * TOK               # v (+ones col): [512 tok, 16 heads * 65]
KV_ROW = KV_KN + KV_V           # 1056768 per core
KV_GROUPS = [[0, 1, 2, 3], [4, 5, 6, 7]]


DBG = False


def _act_raw(nc, out, in_, func, scale, bias):
    """Emit InstActivation directly (the bass wrapper blocks Rsqrt)."""
    eng = nc.scalar
    ins = [eng.lower_ap(in_)]
    for arg in (bias, scale, 0.0):
        if isinstance(arg, (int, float)):
            ins.append(mybir.ImmediateValue(dtype=mybir.dt.float32,
                                            value=float(arg)))
        else:
            ins.append(eng.lower_ap(arg))
    return eng.add_instruction(mybir.InstActivation(
        name=nc.get_next_instruction_name(), func=func, ins=ins,
        outs=[eng.lower_ap(out)]))


def _ap(t, offset, dims):
    return bass.AP(tensor=t, offset=offset, ap=[list(d) for d in dims])


def build_program(reps=1, nocoll_tail=False):
    nc = bacc.Bacc("TRN2", target_bir_lowering=False, debug=False,
                   num_devices=N_CORES)

    xT = nc.declare_dram_parameter("xT", [D, TOK], FP32, isOutput=False)
    csh = nc.declare_dram_parameter("csh", [D, 1], FP32, isOutput=False)
    # per-core quarter of adaln_w (columns [1536*(core%4), +1536))
    aw = nc.declare_dram_parameter("aw", [D, 6 * D // 4], FP8, isOutput=False)
    ab = nc.declare_dram_parameter("ab", [6 * D], FP32, isOutput=False)
    qkvw = nc.declare_dram_parameter("qkvw", [D, 3 * D], FP8, isOutput=False)
    projw = nc.declare_dram_parameter("projw", [D, D], FP8, isOutput=False)
    projb = nc.declare_dram_parameter("projb", [D], FP32, isOutput=False)
    w1 = nc.declare_dram_parameter("w1", [D, HM], FP8, isOutput=False)
    w3 = nc.declare_dram_parameter("w3", [D, HM], FP8, isOutput=False)
    w2 = nc.declare_dram_parameter("w2", [HM, D], FP8, isOutput=False)
    cosq = nc.declare_dram_parameter("cosq", [128, TOK], BF16, isOutput=False)
    sinq = nc.declare_dram_parameter("sinq", [128, TOK], BF16, isOutput=False)
    cosk = nc.declare_dram_parameter("cosk", [128, TOK], BF16, isOutput=False)
    sink = nc.declare_dram_parameter("sink", [128, TOK], BF16, isOutput=False)
    perm = nc.declare_dram_parameter("perm", [128, 128], FP32, isOutput=False)
    consts = nc.declare_dram_parameter("consts", [128, 4], FP32, isOutput=False)
    bcast2 = nc.declare_dram_parameter("bcast2", [3, 128], FP32, isOutput=False)
    outT = nc.declare_dram_parameter("outT", [D, TOK], FP32, isOutput=True)
    if DBG:
        dbg_mods = nc.declare_dram_parameter("dbg_mods", [128, 48], FP32,
                                             isOutput=True)
        dbg_xmod = nc.declare_dram_parameter("dbg_xmod", [128, TOK], FP8,
                                             isOutput=True)
        dbg_kn = nc.declare_dram_parameter("dbg_kn", [128, TOK], FP8,
                                           isOutput=True)
        dbg_qn = nc.declare_dram_parameter("dbg_qn", [128, TOK], FP8,
                                           isOutput=True)
        dbg_den = nc.declare_dram_parameter("dbg_den", [1, TOK], FP32,
                                            isOutput=True)
        dbg_attn = nc.declare_dram_parameter("dbg_attn", [64, TOK], FP8,
                                             isOutput=True)
        dbg_x1 = nc.declare_dram_parameter("dbg_x1", [128, TOK], FP32,
                                           isOutput=True)

    io = locals()
    with tile.TileContext(nc) as tc:
        for _rep in range(reps):
            _body(nc, tc, io, skip_collectives=(nocoll_tail and _rep > 0))
    nc.compile()
    return nc


def _body(nc, tc, io, skip_collectives=False):
    xT, csh, aw, ab = io["xT"], io["csh"], io["aw"], io["ab"]
    qkvw, projw, projb = io["qkvw"], io["projw"], io["projb"]
    w1, w3, w2 = io["w1"], io["w3"], io["w2"]
    cosq, sinq, cosk, sink = io["cosq"], io["sinq"], io["cosk"], io["sink"]
    perm, consts, outT = io["perm"], io["consts"], io["outT"]

    from contextlib import ExitStack
    ctx = ExitStack()
    # pools alive for the whole kernel
    pp = ctx.enter_context(tc.tile_pool(name="persist", bufs=1))
    dr = ctx.enter_context(tc.tile_pool(name="dram", bufs=1, space="DRAM"))

    # ---------- tiny P0 inputs first, then x, then weights ----------
    csh_sb = pp.tile([128, 8, 1], FP32, name="csh_sb")
    nc.sync.dma_start(out=csh_sb,
                      in_=csh.rearrange("(kt p) o -> p kt o", p=128))
    xT_sb = pp.tile([128, KT, TOK], FP32, name="xT_sb")
    for kt in range(KT):
        nc.sync.dma_start(out=xT_sb[:, kt, :],
                          in_=xT[128 * kt:128 * (kt + 1), :])

    # whole-weight SBUF preloads, spread across the 3 DGE queues
    # (aw + qkvw go on the scalar queue inside scope A, aw first)
    w1_sb = pp.tile([128, KT2, 2, HM], FP8, name="w1_sb")
    nc.gpsimd.dma_start(
        out=w1_sb,
        in_=w1.rearrange("(kt2 two p) n -> p kt2 two n", p=128, two=2))
    w3_sb = pp.tile([128, KT2, 2, HM], FP8, name="w3_sb")
    nc.gpsimd.dma_start(
        out=w3_sb,
        in_=w3.rearrange("(kt2 two p) n -> p kt2 two n", p=128, two=2))
    w2_sb = pp.tile([128, HMT2, 2, D], FP8, name="w2_sb")
    nc.gpsimd.dma_start(
        out=w2_sb,
        in_=w2.rearrange("(hm2 two p) n -> p hm2 two n", p=128, two=2))
    projw_sb = pp.tile([64, 8, 2, D], FP8, name="projw_sb")
    nc.gpsimd.dma_start(
        out=projw_sb,
        in_=projw.rearrange("(hp two p) m -> p hp two m", p=64, two=2))

    # ---------- global constants ----------
    consts_sb = pp.tile([128, 4], FP32R, name="consts_sb")
    nc.sync.dma_start(out=consts_sb, in_=consts[:, :].bitcast(FP32R))
    perm_sb = pp.tile([128, 128], FP32R, name="perm_sb")
    nc.sync.dma_start(out=perm_sb, in_=perm[:, :].bitcast(FP32R))
    projb_sb = pp.tile([128, 8], FP32, name="projb_sb")
    nc.sync.dma_start(out=projb_sb,
                      in_=projb.rearrange("(oc p) -> p oc", p=128))
    cosq_sb = pp.tile([128, TOK], BF16, name="cosq_sb")
    sinq_sb = pp.tile([128, TOK], BF16, name="sinq_sb")
    cosk_sb = pp.tile([128, TOK], BF16, name="cosk_sb")
    sink_sb = pp.tile([128, TOK], BF16, name="sink_sb")
    nc.sync.dma_start(out=cosq_sb, in_=cosq[:, :])
    nc.sync.dma_start(out=sinq_sb, in_=sinq[:, :])
    nc.sync.dma_start(out=cosk_sb, in_=cosk[:, :])
    nc.sync.dma_start(out=sink_sb, in_=sink[:, :])
    eps1 = pp.tile([128, 1], FP32, name="eps1")
    nc.vector.memset(eps1, EPS)
    epsd = pp.tile([128, 1], FP32, name="epsd")
    nc.vector.memset(epsd, DH * EPS)
    ebias = pp.tile([128, 1], FP32, name="ebias")
    nc.vector.memset(ebias, EXP_BIAS)
    # ones patterns for fp32r partition-broadcast matmuls (host-prepared)
    bc1 = pp.tile([1, 128], FP32R, name="bc1")
    nc.sync.dma_start(out=bc1, in_=io["bcast2"][0:1, :].bitcast(FP32R))
    bc2 = pp.tile([2, 128], FP32R, name="bc2")
    nc.sync.dma_start(out=bc2, in_=io["bcast2"][1:3, :].bitcast(FP32R))

    mods_sb = pp.tile([128, 6, 8], FP32, name="mods_sb")
    s1p_msa = pp.tile([128, 8], FP32, name="s1p_msa")
    s1p_mlp = pp.tile([128, 8], FP32, name="s1p_mlp")
    bgp = pp.tile([128, 8], FP32, name="bgp")
    gproj = pp.tile([128, 8], FP32, name="gproj")   # gate_msa / S_PROJ
    gmlp = pp.tile([128, 8], FP32, name="gmlp")     # gate_mlp / (S_W3*S_W2)
    x1T = pp.tile([128, KT, TOK], FP32, name="x1T")

    # ---------- DRAM scratch ----------
    mods_q = dr.tile([6 * D // 4], FP32, name="mods_q")
    mods_in = dr.tile([6 * D], FP32, name="mods_in")
    kv_in = dr.tile([KV_ROW], FP8, name="kv_in")
    kv_out = dr.tile([4, KV_ROW], FP8, name="kv_out")

    def rms_bcast(pool_ps, pool_t, rvr, src_tile3, scale, bias, tag):
        """token-wise 1/sqrt(mean(sq)+eps) over the partition (feature) dim,
        via a single ACT Rsqrt, broadcast to all 128 partitions with a tiny
        fp32r ones-matmul on PE."""
        ps_ssq = pool_ps.tile([1, 512], FP32, tag=f"ps_ssq_{tag}", bufs=1,
                              name=f"ps_ssq_{tag}")
        for kt in range(KT):
            xsq = pool_t.tile([128, TOK], FP32R, tag=f"sq_{tag}", bufs=2,
                              name=f"sq_{tag}")
            nc.gpsimd.tensor_mul(xsq, src_tile3[:, kt, :], src_tile3[:, kt, :])
            nc.tensor.matmul(ps_ssq, consts_sb[:, 0:1], xsq,
                             start=(kt == 0), stop=(kt == KT - 1))
        _act_raw(nc, rvr[0:1, :], ps_ssq, AF.Rsqrt, scale, bias)
        ps_rb = pool_ps.tile([128, 512], FP32, tag=f"ps_rb_{tag}", bufs=1,
                             name=f"ps_rb_{tag}")
        nc.tensor.matmul(ps_rb, bc1, rvr[0:1, :],
                         start=True, stop=True)
        return ps_rb

    # =========================================================
    # Scope AB: qn lives from P3 into attention
    # =========================================================
    ab_pool = ctx.enter_context(tc.tile_pool(name="scope_ab", bufs=1))
    vfa_pool = ctx.enter_context(tc.tile_pool(name="vfa", bufs=1))
    qn = ab_pool.tile([128, 8, TOK], FP8, name="qn")
    rvA = ab_pool.tile([2, 512], FP32, name="rvA")
    rvB = ab_pool.tile([2, 512], FP32, name="rvB")
    rvrA = ab_pool.tile([2, 512], FP32R, name="rvrA")
    rvrB = ab_pool.tile([2, 512], FP32R, name="rvrB")

    with tc.tile_pool(name="sA", bufs=1) as pa, \
         tc.tile_pool(name="tA", bufs=2) as tp:

        # ---------- P0: AdaLN mods ----------
        aw_sb = pa.tile([128, KT2, 2, 6 * D // 4], FP8, name="aw_sb")
        nc.scalar.dma_start(
            out=aw_sb,
            in_=aw.rearrange("(kt2 two p) n -> p kt2 two n", p=128, two=2))
        qkvw_sb = pp.tile([128, KT2, 2, 3 * D], FP8, name="qkvw_sb")
        nc.scalar.dma_start(
            out=qkvw_sb,
            in_=qkvw.rearrange("(kt2 two p) n -> p kt2 two n", p=128, two=2))
        with tc.tile_pool(name="psA1", bufs=1, space="PSUM") as ps1:
            # fp8 silu(c), padded so the DoubleRow plane stride is 16B
            silu_c = pa.tile([128, 8, 16], FP8, name="silu_c")
            nc.scalar.activation(out=silu_c[:, :, 0:1], in_=csh_sb,
                                 func=AF.Silu)
            for ncn in range(3):
                ps_m = ps1.tile([1, 512], FP32, tag="ps_mods", bufs=2,
                                name="ps_m")
                for kt2 in range(KT2):
                    nc.tensor.matmul(ps_m, silu_c[:, 2 * kt2:2 * kt2 + 2, 0:1],
                                     aw_sb[:, kt2, :,
                                           512 * ncn:512 * (ncn + 1)],
                                     start=(kt2 == 0), stop=(kt2 == KT2 - 1),
                                     perf_mode=DR)
                stg = rvB[0:1, :]
                nc.vector.tensor_scalar_mul(stg, ps_m, 1.0 / S_AW)
                nc.sync.dma_start(
                    out=_ap(mods_q.tensor, mods_q.offset + 512 * ncn,
                            [[512, 1], [1, 512]]),
                    in_=stg)
            if not skip_collectives:
                nc.gpsimd.collective_compute(
                    "AllGather", mybir.AluOpType.bypass,
                    replica_groups=KV_GROUPS,
                    ins=[mods_q[:]],
                    outs=[mods_in[:]])
            nc.sync.dma_start(
                out=mods_sb,
                in_=mods_in.rearrange("(v kt p) -> p v kt", p=128, kt=8))
            ab_sb = pa.tile([128, 6, 8], FP32, name="ab_sb")
            nc.sync.dma_start(
                out=ab_sb, in_=ab.rearrange("(v kt p) -> p v kt", p=128, kt=8))
            nc.vector.tensor_add(mods_sb, mods_sb, ab_sb)
            nc.scalar.add(s1p_msa, mods_sb[:, 1, :], 1.0)
            nc.scalar.add(s1p_mlp, mods_sb[:, 4, :], 1.0)
            if DBG:
                nc.sync.dma_start(
                    out=io["dbg_mods"][:, :],
                    in_=mods_sb.rearrange("p v kt -> p (v kt)"))
            nc.vector.tensor_mul(bgp, projb_sb, mods_sb[:, 2, :])
            nc.vector.tensor_scalar_mul(gproj, mods_sb[:, 2, :],
                                        1.0 / (S_QKV * S_PROJ / S_QKV))
            nc.vector.tensor_scalar_mul(gmlp, mods_sb[:, 5, :],
                                        1.0 / (S_W3 * S_W2))

            # ---------- P1: x_modT (fp8) ----------
            ps_rb1 = rms_bcast(ps1, tp, rvrA, xT_sb, 1.0 / D,
                               eps1[0:1, :], "n1")
            x_modT = pa.tile([128, KT, TOK], FP8, name="x_modT")
            for kt in range(KT):
                xr = tp.tile([128, TOK], FP32, tag="xr1", name="xr")
                nc.vector.tensor_mul(xr, xT_sb[:, kt, :], ps_rb1)
                nc.vector.tensor_scalar(
                    out=x_modT[:, kt, :], in0=xr,
                    scalar1=s1p_msa[:, kt:kt + 1],
                    scalar2=mods_sb[:, 0, kt:kt + 1],
                    op0=mybir.AluOpType.mult, op1=mybir.AluOpType.add)
            if DBG:
                nc.sync.dma_start(out=io["dbg_xmod"][:, :], in_=x_modT[:, 0, :])

        # ---------- P2/P3: q/k/v projections, rope, kv allgather ----------
        with tc.tile_pool(name="psA2", bufs=1, space="PSUM") as ps2:

            def proj_T(col0, oc):
                ps_p = ps2.tile([128, 512], FP32, tag="ps_pT", bufs=2,
                                name="ps_p")
                for kt2 in range(KT2):
                    nc.tensor.matmul(
                        ps_p,
                        qkvw_sb[:, kt2, :,
                                col0 + 128 * oc: col0 + 128 * (oc + 1)],
                        x_modT[:, 2 * kt2:2 * kt2 + 2, :],
                        start=(kt2 == 0), stop=(kt2 == KT2 - 1),
                        perf_mode=DR)
                return ps_p

            def rope(ps_raw, cos_sb, sin_sb, tag):
                raw = tp.tile([128, TOK], FP32R, tag="raw", bufs=2,
                              name="raw")
                nc.vector.tensor_copy(out=raw, in_=ps_raw)
                ps_sh = ps2.tile([128, 512], FP32, tag="ps_sh", bufs=2,
                                 name="ps_sh")
                nc.tensor.matmul(ps_sh, perm_sb, raw, start=True, stop=True)
                t1 = tp.tile([128, TOK], BF16, tag="t1", bufs=2,
                             name="t1")
                nc.gpsimd.tensor_mul(t1, raw, cos_sb)
                t2 = tp.tile([128, TOK], BF16, tag="t2", bufs=2,
                             name="t2")
                nc.vector.tensor_mul(t2, ps_sh, sin_sb)
                return raw, t1, t2

            def head_rms_bcast(raw, scale, bias, rvr):
                sq = tp.tile([128, TOK], FP32R, tag="hsq", bufs=2,
                             name="sq")
                nc.gpsimd.tensor_mul(sq, raw, raw)
                ps_h = ps2.tile([2, 512], FP32, tag="ps_h", bufs=2,
                                name="ps_h")
                nc.tensor.matmul(ps_h, consts_sb[:, 1:3], sq,
                                 start=True, stop=True)
                _act_raw(nc, rvr[0:2, :], ps_h, AF.Rsqrt, scale, bias)
                ps_b = ps2.tile([128, 512], FP32, tag="ps_b", bufs=2,
                                name="ps_b")
                nc.tensor.matmul(ps_b, bc2, rvr[0:2, :],
                                 start=True, stop=True)
                return ps_b

            def v_chunk(ncn, mt):
                vaug = tp.tile([128, 8, 65], FP8, tag="vaug", bufs=2,
                               name="vaug")
                nc.vector.memset(vaug[:, :, 64:65], 1.0)
                ps_v = ps2.tile([128, 512], FP32, tag="ps_pT", bufs=2,
                                name="ps_v")
                for kt2 in range(KT2):
                    nc.tensor.matmul(
                        ps_v,
                        x_modT[:, 2 * kt2:2 * kt2 + 2,
                               128 * mt:128 * (mt + 1)],
                        qkvw_sb[:, kt2, :,
                                2 * D + 512 * ncn: 2 * D + 512 * (ncn + 1)],
                        start=(kt2 == 0), stop=(kt2 == KT2 - 1),
                        perf_mode=DR)
                nc.vector.tensor_scalar_mul(
                    vaug[:, :, 0:64],
                    ps_v.rearrange("p (h d) -> p h d", d=64),
                    1.0 / S_QKV)
                nc.sync.dma_start(
                    out=_ap(kv_in.tensor,
                            kv_in.offset + KV_KN + 128 * mt * 1040
                            + 65 * 8 * ncn,
                            [[1040, 128], [1, 520]]),
                    in_=vaug.rearrange("p h d -> p (h d)"))

            # k chains with v chunks interleaved (v keeps the PE warm while
            # each k chain round-trips through Pool/ACT/DVE)
            for oc in range(8):
                ps_k = proj_T(D, oc)
                raw, t1, t2 = rope(ps_k, cosk_sb, sink_sb, "k")
                ps_kb = head_rms_bcast(raw, 1.0 / DH, eps1[0:2, :],
                                       rvrA if oc % 2 == 0 else rvrB)
                knt = tp.tile([128, TOK], BF16, tag="knt", bufs=2, name="knt")
                nc.gpsimd.tensor_add(knt, t1, t2)
                kn8 = tp.tile([128, TOK], FP8, tag="kn8", bufs=2, name="kn8")
                nc.vector.tensor_mul(kn8, knt, ps_kb)
                nc.sync.dma_start(
                    out=_ap(kv_in.tensor, kv_in.offset + 128 * oc * 512,
                            [[512, 128], [1, 512]]),
                    in_=kn8)
                if DBG and oc == 0:
                    nc.sync.dma_start(out=io["dbg_kn"][:, :], in_=kn8)
                v_chunk(oc // 4, oc % 4)

            if not skip_collectives:
                nc.gpsimd.collective_compute(
                    "AllGather", mybir.AluOpType.bypass,
                    replica_groups=KV_GROUPS,
                    ins=[kv_in[:]],
                    outs=[kv_out.rearrange("a b -> (a b)")])

            # first half of v (heads 0-7) as soon as the collective lands,
            # overlapping the q loop; 528-wide so the DR plane stride is
            # 16B-aligned
            vfull_a = vfa_pool.tile([128, 4, 4, 528], FP8, name="vfull_a")
            for s2 in range(4):
                eng = nc.sync if s2 % 2 == 0 else nc.scalar
                eng.dma_start(
                    out=vfull_a[:, s2, :, 0:520],
                    in_=_ap(kv_out.tensor,
                            kv_out.offset + s2 * KV_ROW + KV_KN,
                            [[1040, 128], [128 * 1040, 4], [1, 520]]))

            # ---------- P3: qT ----------
            for oc in range(8):
                ps_q = proj_T(0, oc)
                raw, t1, t2 = rope(ps_q, cosq_sb, sinq_sb, "q")
                ps_qb = head_rms_bcast(raw, 1.0, epsd[0:2, :],
                                       rvrA if oc % 2 == 0 else rvrB)
                t3 = tp.tile([128, TOK], BF16, tag="t3_q", bufs=2, name="t3")
                nc.gpsimd.tensor_add(t3, t1, t2)
                nc.vector.tensor_mul(qn[:, oc, :], t3, ps_qb)
                if DBG and oc == 0:
                    nc.sync.dma_start(out=io["dbg_qn"][:, :], in_=qn[:, 0, :])

    # =========================================================
    # Scope B: attention + proj
    # =========================================================
    with tc.tile_pool(name="sB", bufs=1) as pb, \
         tc.tile_pool(name="wB", bufs=2) as wpb, \
         tc.tile_pool(name="tB", bufs=2) as tpb:

        attn_all = pb.tile([64, H, TOK], FP8, name="attn_all")

        kv_t = kv_out.tensor
        kv_off = kv_out.offset
        vfull_b = pb.tile([128, 4, 4, 528], FP8, name="vfull_b")
        for s2 in range(4):
            eng = nc.sync if s2 % 2 == 0 else nc.scalar
            eng.dma_start(
                out=vfull_b[:, s2, :, 0:520],
                in_=_ap(kv_t, kv_off + s2 * KV_ROW + KV_KN + 520,
                        [[1040, 128], [128 * 1040, 4], [1, 520]]))

        def vslice(h, psl, pi2):
            vt, c = (vfull_a, 65 * h) if h < 8 else (vfull_b, 65 * (h - 8))
            return vt[:, psl, 2 * pi2:2 * pi2 + 2, c:c + 65]
        with tc.tile_pool(name="psB1", bufs=1, space="PSUM") as psb:
            for hp in range(8):
                kn_pair = wpb.tile([128, 4, 512], FP8, tag="kn_pair", bufs=3,
                                   name="kn_pair")
                nc.gpsimd.dma_start(
                    out=kn_pair,
                    in_=_ap(kv_t, kv_off + 128 * hp * 512,
                            [[512, 128], [KV_ROW, 4], [1, 512]]))
                ps_o = []
                for hh in range(2):
                    pso = psb.tile([65, 512], FP32, tag=f"ps_o{hh}", bufs=1,
                                   name=f"ps_o{hh}")
                    ps_o.append(pso)
                exp_hist = []
                for sp_i in range(8):
                    s, i2 = sp_i // 2, sp_i % 2
                    exp8 = []
                    for hh in range(2):
                        e8 = tpb.tile([128, 2, 512], FP8, tag=f"exp{hh}",
                                      bufs=4, name=f"exp{hh}")
                        exp8.append(e8)
                    for j in range(2):
                        u = 2 * i2 + j
                        for hh in range(2):
                            ps_s = psb.tile([128, 512], FP32, tag=f"ps_s{hh}",
                                            bufs=3, name=f"ps_s{hh}")
                            nc.tensor.matmul(
                                ps_s,
                                kn_pair[64 * hh:64 * (hh + 1), s,
                                        128 * u:128 * (u + 1)],
                                qn[64 * hh:64 * (hh + 1), hp, :],
                                start=True, stop=True,
                                tile_position=(64 * hh, 0))
                            nc.scalar.activation(out=exp8[hh][:, j, :],
                                                 in_=ps_s,
                                                 func=AF.Exp, bias=ebias[:, :])
                    exp_hist.append((sp_i, exp8))
                    # lag the PV accumulation by one step so it never parks
                    # the in-order PE queue behind the exps it consumes
                    if len(exp_hist) >= 2:
                        pi, pexp = exp_hist.pop(0)
                        psl, pi2 = pi // 2, pi % 2
                        for hh in range(2):
                            nc.tensor.matmul(
                                ps_o[hh],
                                vslice(2 * hp + hh, psl, pi2),
                                pexp[hh],
                                start=(pi == 0), stop=False,
                                perf_mode=DR)
                pi, pexp = exp_hist.pop(0)
                psl, pi2 = pi // 2, pi % 2
                for hh in range(2):
                    nc.tensor.matmul(
                        ps_o[hh],
                        vslice(2 * hp + hh, psl, pi2),
                        pexp[hh],
                        start=False, stop=True,
                        perf_mode=DR)
                for hh in range(2):
                    h = 2 * hp + hh
                    rd = (rvA if hh == 0 else rvB)[0:1, :]
                    nc.vector.reciprocal(out=rd, in_=ps_o[hh][64:65, :])
                    rdr = (rvrA if hh == 0 else rvrB)[0:1, :]
                    nc.vector.tensor_copy(out=rdr, in_=rd)
                    ps_db = psb.tile([128, 512], FP32, tag=f"ps_s{hh}",
                                     bufs=3, name="ps_db")
                    nc.tensor.matmul(ps_db[0:64, :], bc1[0:1, 0:64], rdr,
                                     start=True, stop=True)
                    rdb = tpb.tile([64, 512], BF16, tag=f"rdb{hh}", bufs=2,
                                   name=f"rdb{hh}")
                    nc.vector.tensor_copy(out=rdb, in_=ps_db[0:64, :])
                    nc.vector.tensor_mul(attn_all[:, h, :],
                                         ps_o[hh][0:64, :], rdb)
                    if DBG and h == 0:
                        nc.sync.dma_start(out=io["dbg_den"][:, :], in_=rd)
                        nc.sync.dma_start(out=io["dbg_attn"][:, :],
                                          in_=attn_all[:, 0, :])

        # ---------- P5: proj + gated residual -> x1T ----------
        with tc.tile_pool(name="psB2", bufs=1, space="PSUM") as psb2:
            for oc in range(8):
                ps_p = psb2.tile([128, 512], FP32, tag="ps_proj", bufs=2,
                                 name="ps_p")
                for hp in range(8):
                    nc.tensor.matmul(ps_p,
                                     projw_sb[:, hp, :,
                                              128 * oc:128 * (oc + 1)],
                                     attn_all[:, 2 * hp:2 * hp + 2, :],
                                     start=(hp == 0), stop=(hp == 7),
                                     perf_mode=DR)
                t1 = tpb.tile([128, TOK], FP32, tag="t1_proj", bufs=2,
                              name="t1")
                nc.vector.tensor_scalar(
                    out=t1, in0=ps_p,
                    scalar1=gproj[:, oc:oc + 1], scalar2=bgp[:, oc:oc + 1],
                    op0=mybir.AluOpType.mult, op1=mybir.AluOpType.add)
                nc.gpsimd.tensor_add(x1T[:, oc, :], t1, xT_sb[:, oc, :])
                if DBG and oc == 0:
                    nc.sync.dma_start(out=io["dbg_x1"][:, :], in_=x1T[:, 0, :])

    # =========================================================
    # Scope C: norm2 + MLP
    # =========================================================
    with tc.tile_pool(name="sC", bufs=1) as pc, \
         tc.tile_pool(name="tC", bufs=2) as tpc, \
         tc.tile_pool(name="smC", bufs=1) as spc, \
         tc.tile_pool(name="psC", bufs=1, space="PSUM") as psc:

        rvrC = pc.tile([2, 512], FP32R, name="rvrC")
        ps_rb2 = rms_bcast(psc, tpc, rvrC, x1T, 1.0 / D,
                           eps1[0:1, :], "n2")
        x1_modT = pc.tile([128, KT, TOK], FP8, name="x1_modT")
        for kt in range(KT):
            xr2 = tpc.tile([128, TOK], BF16, tag="xr2", name="xr2")
            nc.vector.tensor_mul(xr2, x1T[:, kt, :], ps_rb2)
            nc.vector.tensor_scalar(
                out=x1_modT[:, kt, :], in0=xr2,
                scalar1=s1p_mlp[:, kt:kt + 1],
                scalar2=mods_sb[:, 3, kt:kt + 1],
                op0=mybir.AluOpType.mult, op1=mybir.AluOpType.add)

        mT = pc.tile([128, HMT, TOK], FP8, name="mT")
        for hm in range(HMT):
            ps_u = psc.tile([128, 512], FP32, tag="ps_u", bufs=2, name="ps_u")
            ps_g = psc.tile([128, 512], FP32, tag="ps_g", bufs=2, name="ps_g")
            for kt2 in range(KT2):
                nc.tensor.matmul(ps_u,
                                 w1_sb[:, kt2, :, 128 * hm:128 * (hm + 1)],
                                 x1_modT[:, 2 * kt2:2 * kt2 + 2, :],
                                 start=(kt2 == 0), stop=(kt2 == KT2 - 1),
                                 perf_mode=DR)
            for kt2 in range(KT2):
                nc.tensor.matmul(ps_g,
                                 w3_sb[:, kt2, :, 128 * hm:128 * (hm + 1)],
                                 x1_modT[:, 2 * kt2:2 * kt2 + 2, :],
                                 start=(kt2 == 0), stop=(kt2 == KT2 - 1),
                                 perf_mode=DR)
            tsil = tpc.tile([128, TOK], BF16, tag="tsil", name="tsil")
            nc.scalar.activation(out=tsil, in_=ps_u, func=AF.Silu,
                                 scale=1.0 / S_W1)
            nc.vector.tensor_mul(mT[:, hm, :], tsil, ps_g)

        for oc in range(8):
            ps_w2 = psc.tile([128, 512], FP32, tag="ps_w2", bufs=2,
                             name="ps_w2")
            for hm2 in range(HMT2):
                nc.tensor.matmul(ps_w2,
                                 w2_sb[:, hm2, :, 128 * oc:128 * (oc + 1)],
                                 mT[:, 2 * hm2:2 * hm2 + 2, :],
                                 start=(hm2 == 0), stop=(hm2 == HMT2 - 1),
                                 perf_mode=DR)
            t3 = tpc.tile([128, TOK], BF16, tag="t3_out", bufs=2, name="t3")
            nc.vector.tensor_scalar(
                out=t3, in0=ps_w2,
                scalar1=gmlp[:, oc:oc + 1], scalar2=None,
                op0=mybir.AluOpType.mult)
            outf = tpc.tile([128, TOK], FP32, tag="outf", bufs=2, name="outf")
            nc.gpsimd.tensor_add(outf, t3, x1T[:, oc, :])
            nc.sync.dma_start(out=outT[128 * oc:128 * (oc + 1), :], in_=outf)

    ctx.close()


# ------------------------------------------------------------------
# host side
# ------------------------------------------------------------------

def _to_fp8(a, scale):
    import ml_dtypes
    return np.clip(np.asarray(a, np.float32) * scale, -240.0, 240.0).astype(
        ml_dtypes.float8_e4m3)


def _host_tables(pos, lnq_w, lnk_w):
    half = DH // 2
    freqs = (1.0 / (10000.0 ** (np.arange(half, dtype=np.float32) / half))
             ).astype(np.float32)
    ang = pos.astype(np.float32)[:, None] * freqs[None, :]      # [T, 32]
    cos2 = np.concatenate([np.cos(ang), np.cos(ang)], -1).astype(np.float32)
    sin2 = np.concatenate([np.sin(ang), np.sin(ang)], -1).astype(np.float32)
    shufsrc = np.concatenate([np.arange(32) + 32, np.arange(32)])
    cosF_q = cos2 * lnq_w[None, :]
    sinF_q = sin2 * lnq_w[shufsrc][None, :]
    cosF_k = cos2 * lnk_w[None, :]
    sinF_k = sin2 * lnk_w[shufsrc][None, :]

    P = np.zeros((128, 128), np.float32)
    for blk in (0, 64):
        for m in range(64):
            P[blk + shufsrc[m], blk + m] = -1.0 if m < 32 else 1.0

    consts = np.zeros((128, 4), np.float32)
    consts[:, 0] = 1.0
    consts[0:64, 1] = 1.0
    consts[64:128, 2] = 1.0
    bcast2_host = np.zeros((3, 128), np.float32)
    bcast2_host[0, :] = 1.0
    bcast2_host[1, 0:64] = 1.0
    bcast2_host[2, 64:128] = 1.0
    return cosF_q, sinF_q, cosF_k, sinF_k, P, consts, bcast2_host


def _prep_in_maps(inputs):
    x = np.asarray(inputs["x"], np.float32)
    c = np.asarray(inputs["c"], np.float32)
    pos = np.asarray(inputs["pos"])
    cosF_q, sinF_q, cosF_k, sinF_k, P, consts, bcast2_host = _host_tables(
        pos, np.asarray(inputs["lnq_w"], np.float32),
        np.asarray(inputs["lnk_w"], np.float32))
    shared = {
        "ab": np.ascontiguousarray(inputs["adaln_b"], np.float32),
        "qkvw": _to_fp8(inputs["qkv_w"], S_QKV),
        "projw": _to_fp8(inputs["proj_w"], S_PROJ),
        "projb": np.ascontiguousarray(inputs["proj_b"], np.float32),
        "w1": _to_fp8(inputs["w1_w"], S_W1),
        "w3": _to_fp8(inputs["w3_w"], S_W3),
        "w2": _to_fp8(inputs["w2_w"], S_W2),
        "perm": P, "consts": consts, "bcast2": bcast2_host,
    }
    aw8 = _to_fp8(inputs["adaln_w"], S_AW)
    in_maps = []
    for core in range(N_CORES):
        b, ti = core // 4, core % 4
        q0 = TOK * ti
        import ml_dtypes
        tile2 = lambda a: np.ascontiguousarray(
            np.tile(a[q0:q0 + TOK].T, (2, 1))).astype(
                ml_dtypes.bfloat16)  # [64,512] -> [128,512]
        m = dict(shared)
        m["xT"] = np.ascontiguousarray(x[b, q0:q0 + TOK, :].T)
        m["csh"] = np.ascontiguousarray(c[b]).reshape(D, 1)
        m["aw"] = np.ascontiguousarray(
            aw8[:, 1536 * ti:1536 * (ti + 1)])
        m["cosq"] = tile2(cosF_q)
        m["sinq"] = tile2(sinF_q)
        m["cosk"] = tile2(cosF_k)
        m["sink"] = tile2(sinF_k)
        in_maps.append(m)
    return in_maps


_RUNNER = {}


def _get_runner(reps=1, nocoll_tail=False):
    global _RUNNER
    key = (reps, nocoll_tail)
    if key in _RUNNER:
        return _RUNNER[key]
    import jax
    from jax.sharding import Mesh, PartitionSpec
    from jax.experimental.shard_map import shard_map
    from concourse import bass2jax, mybir as _mybir

    nc = build_program(reps, nocoll_tail)
    bass2jax.install_neuronx_cc_hook()

    partition_name = (nc.partition_id_tensor.name
                      if nc.partition_id_tensor else None)
    in_names, out_names, out_avals, zero_outs = [], [], [], []
    for alloc in nc.m.functions[0].allocations:
        if not isinstance(alloc, _mybir.MemoryLocationSet):
            continue
        name = alloc.memorylocations[0].name
        if alloc.kind == "ExternalInput":
            if name != partition_name:
                in_names.append(name)
        elif alloc.kind == "ExternalOutput":
            shape = tuple(alloc.tensor_shape)
            dtype = _mybir.dt.np(alloc.dtype)
            out_names.append(name)
            out_avals.append(jax.core.ShapedArray(shape, dtype))
            zero_outs.append(np.zeros(shape, dtype))
    n_params = len(in_names)
    n_outs = len(out_avals)
    all_names = in_names + out_names
    if partition_name is not None:
        all_names = all_names + [partition_name]
    donate = tuple(range(n_params, n_params + n_outs))

    def _bd(*args):
        operands = list(args)
        if partition_name is not None:
            operands.append(bass2jax.partition_id_tensor())
        outs = bass2jax._bass_exec_p.bind(
            *operands, out_avals=tuple(out_avals), in_names=tuple(all_names),
            out_names=tuple(out_names), lowering_input_output_aliases=(),
            sim_require_finite=True, sim_require_nnan=True, nc=nc)
        return tuple(outs)

    devices = jax.devices()[:N_CORES]
    mesh = Mesh(np.asarray(devices), ("core",))
    sharded = jax.jit(
        shard_map(_bd, mesh=mesh,
                  in_specs=(PartitionSpec("core"),) * (n_params + n_outs),
                  out_specs=(PartitionSpec("core"),) * n_outs,
                  check_rep=False),
        donate_argnums=donate, keep_unused=True)

    def run(in_maps):
        concat_in = [np.concatenate([np.asarray(m[nm]) for m in in_maps], 0)
                     for nm in in_names]
        concat_zeros = [np.zeros((N_CORES * z.shape[0], *z.shape[1:]), z.dtype)
                        for z in zero_outs]
        out_arrs = sharded(*concat_in, *concat_zeros)
        return [
            {nm: np.asarray(out_arrs[i]).reshape(N_CORES, *out_avals[i].shape)[cc]
             for i, nm in enumerate(out_names)}
            for cc in range(N_CORES)
        ]

    run.nc = nc
    run.in_names = in_names
    run.zero_outs = zero_outs
    run.sharded = sharded
    _RUNNER[key] = run
    return run


def kernel(**inputs) -> np.ndarray:
    run = _get_runner()
    in_maps = _prep_in_maps(inputs)
    results = run(in_maps)
    out = np.empty((B, T, D), np.float32)
    for core in range(N_CORES):
        b, ti = core // 4, core % 4
        out[b, TOK * ti:TOK * (ti + 1), :] = results[core]["outT"].T
    return out
